# revision 1
# baseline (speedup 1.0000x reference)
"""Neural CDE (RK4, piecewise-constant path derivative) Trainium2 kernel.

Strategy: pure batch parallelism, B=128 -> 16 per core across 8 cores.
Per core, the state is kept feature-major in "split form": a [128, 16] SBUF
tile whose semantic value is top[64] + bottom[64]. This lets the x-contraction
of the einsum land as a free-dim reduce and removes all fold ops from the
recurrence (the L1 weight matrix is stored row-duplicated so the matmul folds
the halves for free).

Matmuls run in double-bf16: weights W ~= Wb + Wr (bf16 value + bf16 residual)
and activations h ~= hb + hr likewise; each layer accumulates
Wb.hb + Wb.hr + Wr.hb (+ Wr.hr) in fp32 PSUM. This keeps fp32-grade accuracy
through the 512-step recurrence while using FWL-rate bf16 weight loads (fp32
matmuls lower to two-pass weight loads on TRN2, ~2x slower).

Per RK4 stage:
  MM L1 (W1 split, K=128) -> relu(+b1) -> MM L2 -> relu(+b2) ->
  b3 seed matmuls + 8 accumulating W3-chunk matmul groups (x-major column
  permute) -> tanh -> elementwise * [dt*v spread | -0.001*dt*sum(v)] ->
  contiguous strided reduce -> scalar_tensor_tensor RK4 updates.
"""

import os
import sys
from contextlib import ExitStack

import numpy as np
import ml_dtypes

sys.path.insert(0, "/opt/trn_rl_repo")

import concourse.bass as bass
import concourse.tile as tile
from concourse import bacc
from concourse import mybir
from concourse.bass_utils import run_bass_kernel_spmd

B, L, X, Z, H = 128, 512, 16, 64, 128
NCORES = 8
BPC = B // NCORES  # 16
DT = 0.1
F32 = mybir.dt.float32
BF16 = mybir.dt.bfloat16
AF = mybir.ActivationFunctionType
OP = mybir.AluOpType
NTERM = 4  # 3: drop Wr.hr ; 4: keep it

# x-major permutation: psum position (p, c) holds original W3 column z*16+x
# with x = 2c + (p>=64), z = p%64
_p = np.arange(128)
_c = np.arange(8)
ORIG_COL = (_p[None, :] % 64) * 16 + 2 * _c[:, None] + (_p[None, :] // 64)  # [8,128]


def build_nc(l_steps=L, nterm=NTERM):
    nc = bacc.Bacc("TRN2")

    # ---- DRAM parameters (per core) ----
    dp = nc.declare_dram_parameter
    vsmall = dp("vsmall", [l_steps, 256], F32, isOutput=False).ap()  # dt*v, x-major
    svd = dp("svd", [l_steps, 16], F32, isOutput=False).ap()  # -0.001*dt*sum_x v
    # [w1b|w1r|w2b|w2r|w3b(1024)|w3r(1024)] all bf16, single DMA
    wmm_d = dp("wmm", [128, 2560], BF16, isOutput=False).ap()
    # [b3b|b3r|sel8] single DMA
    b3sel_d = dp("b3sel", [8, 384], BF16, isOutput=False).ap()
    b1_d = dp("b1c", [128, 1], F32, isOutput=False).ap()
    b2_d = dp("b2c", [128, 1], F32, isOutput=False).ap()
    wi1x_d = dp("wi1x", [16, 144], F32, isOutput=False).ap()  # [wi1 | x0t]
    wi2_d = dp("wi2", [128, 128], F32, isOutput=False).ap()
    wi3_d = dp("wi3", [128, 64], F32, isOutput=False).ap()
    bi1_d = dp("bi1", [128, 1], F32, isOutput=False).ap()
    bi2_d = dp("bi2", [128, 1], F32, isOutput=False).ap()
    bi3_d = dp("bi3", [64, 1], F32, isOutput=False).ap()
    # split-form state per step; host folds top+bottom halves
    zall = dp("zall", [l_steps, 128, BPC], F32, isOutput=True).ap()

    with tile.TileContext(nc) as tc, ExitStack() as ctx:
        singles = ctx.enter_context(tc.tile_pool(name="singles", bufs=1))
        wfp = ctx.enter_context(tc.tile_pool(name="wfp", bufs=4))
        gep = ctx.enter_context(tc.tile_pool(name="gep", bufs=6))
        mp = ctx.enter_context(tc.tile_pool(name="mp", bufs=3))
        qp = ctx.enter_context(tc.tile_pool(name="qp", bufs=8))
        hp = ctx.enter_context(tc.tile_pool(name="hp", bufs=6))
        zbp = ctx.enter_context(tc.tile_pool(name="zbp", bufs=6))
        kp = ctx.enter_context(tc.tile_pool(name="kp", bufs=4))
        ph1p = ctx.enter_context(tc.tile_pool(name="ph1p", bufs=2, space="PSUM"))
        ph2p = ctx.enter_context(tc.tile_pool(name="ph2p", bufs=2, space="PSUM"))
        gpp = ctx.enter_context(tc.tile_pool(name="gpp", bufs=2, space="PSUM"))

        dma = nc.sync.dma_start

        def load(pool, ap):
            t = pool.tile(list(ap.shape), ap.dtype, tag=ap.tensor.name)
            dma(out=t[:], in_=ap)
            return t

        wmm = load(singles, wmm_d)
        w1b, w1r = wmm[:, 0:128], wmm[:, 128:256]
        w2b, w2r = wmm[:, 256:384], wmm[:, 384:512]
        w3b, w3r = wmm[:, 512:1536], wmm[:, 1536:2560]
        b3sel = load(singles, b3sel_d)
        b3b, b3r, sel8 = b3sel[:, 0:128], b3sel[:, 128:256], b3sel[:, 256:384]
        b1c = load(singles, b1_d)
        b2c = load(singles, b2_d)
        wi1x = load(singles, wi1x_d)
        wi1, x0t = wi1x[:, 0:128], wi1x[:, 128:144]
        wi2 = load(singles, wi2_d)
        wi3 = load(singles, wi3_d)
        bi1 = load(singles, bi1_d)
        bi2 = load(singles, bi2_d)
        bi3 = load(singles, bi3_d)
        zeros = singles.tile([128, BPC], F32, tag="zeros")
        nc.vector.memset(zeros[:], 0.0)

        mm = nc.tensor.matmul

        # ---- init MLP (fp32): z0 = mlp(x(t0)) ----
        ph_i1 = ph1p.tile([128, BPC], F32, tag="ph1")
        mm(ph_i1[:], wi1, x0t, start=True, stop=True)
        hi1 = singles.tile([128, BPC], F32, tag="hi1")
        nc.scalar.activation(hi1[:], ph_i1[:], AF.Relu, bias=bi1[:])
        ph_i2 = ph2p.tile([128, BPC], F32, tag="ph2")
        mm(ph_i2[:], wi2[:], hi1[:], start=True, stop=True)
        hi2 = singles.tile([128, BPC], F32, tag="hi2")
        nc.scalar.activation(hi2[:], ph_i2[:], AF.Relu, bias=bi2[:])
        ph_i3 = ph1p.tile([64, BPC], F32, tag="ph1")
        mm(ph_i3[:], wi3[:], hi2[:], start=True, stop=True)

        # state tile for step 0: split form [z0; 0]
        ge_cur = gep.tile([128, 160], F32, tag="ge")
        slot_cur = ge_cur[:, 128:144]
        nc.vector.tensor_scalar_add(ge_cur[0:64, 128:144], ph_i3[:], bi3[:])
        nc.vector.memset(ge_cur[64:128, 128:144], 0.0)
        zsb_cur = zbp.tile([128, BPC], BF16, tag="zsb")
        nc.vector.tensor_copy(out=zsb_cur[:], in_=slot_cur)
        zsr_cur = zbp.tile([128, BPC], BF16, tag="zsr")
        nc.vector.scalar_tensor_tensor(
            out=zsr_cur[:], in0=zsb_cur[:], scalar=-1.0, in1=slot_cur,
            op0=OP.mult, op1=OP.add,
        )

        def layer(pool, tag, wb, wr, rb, rr, bias, relu_engine):
            """psum = wb.T rb + wb.T rr + wr.T rb (+ wr.T rr); then relu+bias."""
            ph = pool.tile([128, BPC], F32, tag=tag)
            mm(ph[:], wb, rb[:], start=True, stop=False, skip_group_check=True)
            mm(ph[:], wr, rb[:], start=False, stop=False, skip_group_check=True)
            mm(ph[:], wb, rr[:], start=False, stop=(nterm == 3),
               skip_group_check=True)
            if nterm == 4:
                mm(ph[:], wr, rr[:], start=False, stop=True, skip_group_check=True)
            h32 = hp.tile([128, BPC], F32, tag=tag + "h32")
            if relu_engine == "vector":
                nc.vector.scalar_tensor_tensor(
                    out=h32[:], in0=ph[:], scalar=bias[:], in1=zeros[:],
                    op0=OP.add, op1=OP.max,
                )
            else:
                nc.scalar.activation(h32[:], ph[:], AF.Relu, bias=bias[:])
            hb = hp.tile([128, BPC], BF16, tag=tag + "hb")
            nc.vector.tensor_copy(out=hb[:], in_=h32[:])
            hr = hp.tile([128, BPC], BF16, tag=tag + "hr")
            nc.vector.scalar_tensor_tensor(
                out=hr[:], in0=hb[:], scalar=-1.0, in1=h32[:],
                op0=OP.mult, op1=OP.add,
            )
            return hb, hr

        stage_scale = [0.5, 0.5, 1.0]

        for t in range(l_steps):
            # build wf [128, 144] by replication-DMA from vsmall/svd
            wf = wfp.tile([128, 144], F32, tag="wf")
            vbase = vsmall[t]
            for half in range(2):
                src = bass.AP(
                    tensor=vbase.tensor,
                    offset=vbase.offset + 16 * half,
                    ap=[[0, 64], [32, 8], [1, 16]],
                )
                dst = wf[64 * half:64 * (half + 1), 0:128].rearrange(
                    "p (c j) -> p c j", j=16
                )
                dma(out=dst, in_=src)
            sbase = svd[t]
            src = bass.AP(tensor=sbase.tensor, offset=sbase.offset,
                          ap=[[0, 128], [1, 16]])
            dma(out=wf[:, 128:144], in_=src)

            # output: split-form state at start of step t (host folds halves)
            dma(out=zall[t], in_=ge_cur[:, 128:144])

            qs = []
            ge_s, zsb_s, zsr_s = ge_cur, zsb_cur, zsr_cur
            kacc12 = kacc123 = pfin = None
            for s in range(4):
                gp = gpp.tile([128, 128], F32, tag="gp")
                h1b, h1r = layer(ph1p, "ph1", w1b, w1r, zsb_s, zsr_s, b1c,
                                 "vector")
                # b3 seeds issue while the PE queue would stall on relu1
                mm(gp[:], b3b, sel8, start=True, stop=False,
                   skip_group_check=True)
                mm(gp[:], b3r, sel8, start=False, stop=False,
                   skip_group_check=True)
                h2b, h2r = layer(ph2p, "ph2", w2b, w2r, h1b, h1r, b2c,
                                 "scalar")

                # hb-only terms for every chunk first: the in-order PE queue
                # never stalls on h2r behind a ready hb-term
                for c in range(8):
                    sl = gp[:, c * 16:(c + 1) * 16]
                    wbc = w3b[:, c * 128:(c + 1) * 128]
                    mm(sl, wbc, h2b[:], start=False, stop=False,
                       skip_group_check=True)
                for c in range(8):
                    sl = gp[:, c * 16:(c + 1) * 16]
                    wrc = w3r[:, c * 128:(c + 1) * 128]
                    mm(sl, wrc, h2b[:], start=False, stop=False,
                       skip_group_check=True)
                for c in range(8):
                    sl = gp[:, c * 16:(c + 1) * 16]
                    wbc = w3b[:, c * 128:(c + 1) * 128]
                    wrc = w3r[:, c * 128:(c + 1) * 128]
                    last = c == 7
                    mm(sl, wbc, h2r[:], start=False,
                       stop=(last and nterm == 3), skip_group_check=True)
                    if nterm == 4:
                        mm(sl, wrc, h2r[:], start=False, stop=last,
                           skip_group_check=True)
                nc.scalar.activation(ge_s[:, 0:128], gp[:], AF.Tanh, bias=0.0)
                # m in (j, c)-contiguous layout so the reduce reads unit-stride
                m = mp.tile([128, 144], F32, tag="m")
                nc.vector.tensor_tensor(
                    out=m[:].rearrange("p (j c) -> p j c", c=9),
                    in0=ge_s[:, 0:144].rearrange("p (c j) -> p j c", j=16),
                    in1=wf[:].rearrange("p (c j) -> p j c", j=16),
                    op=OP.mult,
                )
                q = qp.tile([128, BPC], F32, tag="q")
                nc.vector.tensor_reduce(
                    out=q[:], in_=m[:].rearrange("p (j c) -> p j c", c=9),
                    axis=mybir.AxisListType.X, op=OP.add,
                )
                qs.append(q)

                if s < 3:
                    ge_n = gep.tile([128, 160], F32, tag="ge")
                    zsb_n = zbp.tile([128, BPC], BF16, tag="zsb")
                    zsr_n = zbp.tile([128, BPC], BF16, tag="zsr")
                    nc.vector.scalar_tensor_tensor(
                        out=zsb_n[:], in0=q[:], scalar=stage_scale[s],
                        in1=slot_cur, op0=OP.mult, op1=OP.add,
                    )
                    nc.vector.scalar_tensor_tensor(
                        out=ge_n[:, 128:144], in0=q[:], scalar=stage_scale[s],
                        in1=slot_cur, op0=OP.mult, op1=OP.add,
                    )
                    nc.vector.scalar_tensor_tensor(
                        out=zsr_n[:], in0=zsb_n[:], scalar=-1.0,
                        in1=ge_n[:, 128:144], op0=OP.mult, op1=OP.add,
                    )
                    ge_s, zsb_s, zsr_s = ge_n, zsb_n, zsr_n
                if s == 1:
                    kacc12 = kp.tile([128, BPC], F32, tag="k")
                    nc.vector.scalar_tensor_tensor(
                        out=kacc12[:], in0=qs[1][:], scalar=2.0, in1=qs[0][:],
                        op0=OP.mult, op1=OP.add,
                    )
                elif s == 2:
                    kacc123 = kp.tile([128, BPC], F32, tag="k")
                    nc.vector.scalar_tensor_tensor(
                        out=kacc123[:], in0=qs[2][:], scalar=2.0, in1=kacc12[:],
                        op0=OP.mult, op1=OP.add,
                    )
                    pfin = kp.tile([128, BPC], F32, tag="k")
                    nc.vector.scalar_tensor_tensor(
                        out=pfin[:], in0=kacc123[:], scalar=1.0 / 6.0,
                        in1=slot_cur, op0=OP.mult, op1=OP.add,
                    )
                elif s == 3:
                    ge_next = gep.tile([128, 160], F32, tag="ge")
                    zsb_next = zbp.tile([128, BPC], BF16, tag="zsb")
                    zsr_next = zbp.tile([128, BPC], BF16, tag="zsr")
                    nc.vector.scalar_tensor_tensor(
                        out=zsb_next[:], in0=q[:], scalar=1.0 / 6.0,
                        in1=pfin[:], op0=OP.mult, op1=OP.add,
                    )
                    nc.vector.scalar_tensor_tensor(
                        out=ge_next[:, 128:144], in0=q[:], scalar=1.0 / 6.0,
                        in1=pfin[:], op0=OP.mult, op1=OP.add,
                    )
                    nc.vector.scalar_tensor_tensor(
                        out=zsr_next[:], in0=zsb_next[:], scalar=-1.0,
                        in1=ge_next[:, 128:144], op0=OP.mult, op1=OP.add,
                    )
            ge_cur, zsb_cur, zsr_cur = ge_next, zsb_next, zsr_next
            slot_cur = ge_cur[:, 128:144]

    nc.compile()
    return nc


def _split_bf16(w):
    wb = np.asarray(w, np.float32).astype(ml_dtypes.bfloat16)
    wr = (np.asarray(w, np.float32) - wb.astype(np.float32)).astype(
        ml_dtypes.bfloat16)
    return wb, wr


def _prep_inputs(t, x, dyn_w1, dyn_b1, dyn_w2, dyn_b2, dyn_w3, dyn_b3,
                 init_w1, init_b1, init_w2, init_b2, init_w3, init_b3,
                 l_steps=L):
    x = np.asarray(x, dtype=np.float32)
    x_aug = np.concatenate([x, x[:, -1:]], axis=1)
    v = (x_aug[:, 1:] - x_aug[:, :-1]) / DT  # [B, L, X]
    sv = v.sum(-1)  # [B, L]

    w1s = np.concatenate([dyn_w1, dyn_w1], axis=0).astype(np.float32)
    w3x = np.empty((H, 1024), dtype=np.float32)
    for c in range(8):
        w3x[:, c * 128:(c + 1) * 128] = dyn_w3[:, ORIG_COL[c]]
    b3row = np.asarray(dyn_b3, np.float32)[ORIG_COL]  # [8, 128]

    w1b, w1r = _split_bf16(w1s)
    w2b, w2r = _split_bf16(dyn_w2)
    w3b, w3r = _split_bf16(w3x)
    b3b, b3r = _split_bf16(b3row)
    sel8 = np.zeros((8, 128), dtype=ml_dtypes.bfloat16)
    for k in range(8):
        sel8[k, k * 16:(k + 1) * 16] = 1.0

    wmm = np.concatenate([w1b, w1r, w2b, w2r, w3b, w3r], axis=1)  # [128, 2560]
    b3sel = np.concatenate([b3b, b3r, sel8], axis=1)              # [8, 384]

    shared = dict(
        wmm=np.ascontiguousarray(wmm), b3sel=np.ascontiguousarray(b3sel),
        b1c=np.asarray(dyn_b1, np.float32).reshape(128, 1),
        b2c=np.asarray(dyn_b2, np.float32).reshape(128, 1),
        wi2=np.asarray(init_w2, np.float32),
        wi3=np.asarray(init_w3, np.float32),
        bi1=np.asarray(init_b1, np.float32).reshape(128, 1),
        bi2=np.asarray(init_b2, np.float32).reshape(128, 1),
        bi3=np.asarray(init_b3, np.float32).reshape(64, 1),
    )
    wi1 = np.asarray(init_w1, np.float32)

    in_maps = []
    for core in range(NCORES):
        sl = slice(core * BPC, (core + 1) * BPC)
        vb = v[sl, :l_steps]            # [BPC, l, X]
        svb = sv[sl, :l_steps]          # [BPC, l]
        vsm = (DT * vb.transpose(1, 2, 0)).reshape(l_steps, 256).astype(np.float32)
        svdc = (-0.001 * DT * svb.T).astype(np.float32)  # [l, BPC]
        x0tc = x[sl, 0, :].T.astype(np.float32)          # [X, BPC]
        wi1x = np.concatenate([wi1, x0tc], axis=1)       # [16, 144]
        m = dict(shared)
        m.update(vsmall=np.ascontiguousarray(vsm), svd=np.ascontiguousarray(svdc),
                 wi1x=np.ascontiguousarray(wi1x))
        in_maps.append(m)
    return in_maps


_NC_CACHE = {}


def kernel_traced(trace=False, **inputs):
    key = L
    if key not in _NC_CACHE:
        _NC_CACHE[key] = build_nc(L)
    nc = _NC_CACHE[key]
    in_maps = _prep_inputs(**inputs, l_steps=L)
    res = run_bass_kernel_spmd(nc, in_maps, list(range(NCORES)), trace=trace)
    out = np.empty((B, L, Z), dtype=np.float32)
    for core in range(NCORES):
        zall = res.results[core]["zall"]  # [L, 128, BPC] split form
        zf = zall[:, :Z] + zall[:, Z:]
        out[core * BPC:(core + 1) * BPC] = zf.transpose(2, 0, 1)
    return out, res


def kernel(**inputs):
    return kernel_traced(trace=False, **inputs)[0]

